# revision 7
# baseline (speedup 1.0000x reference)
"""Trainium2 Bass kernel for ExplicitRandomWalkEncoder.

Math (reference):
    x_encoded = x @ W_f.T + b_f                      # [N, H]
    feats     = x_encoded[walks]                     # [B, T, H]
    h_T       = GRU(feats)  (torch gate order r,z,n) # [B, H]

Key algebraic fold: the feature encoder commutes into the GRU input
projection, so the device never materializes x_encoded:
    gx = feats @ W_ih.T + b_ih
       = x[walks] @ (W_ih @ W_f).T + (W_ih @ b_f + b_ih)
The kernel gathers raw x rows (512B each) and applies the combined
input projection W_c = W_ih @ W_f.

Sharding: data-parallel over walks. Each of the 8 cores processes
2048 walks; x and all weights are replicated. No collectives.

Per-core per-step dataflow (hidden-major layout, hidden dim on
partitions, batch on free dim):
  1. indirect DMA gathers 2048 rows of x -> G [128 batch-part, 16*128]
  2. PE transposes each [128,128] walk-tile -> xT [128 feat, 2048]
  3. PE matmuls (float32r): per 512-batch chunk, psum_r/z = W_c_g @ xT
     accumulated with W_hh_g @ h; psum_nx, psum_nh separate
  4. ACT: r = sigmoid(psum_r + b_r), z likewise
  5. DVE: rhn = (psum_nh + b_hn) * r ; narg = rhn + psum_nx
  6. ACT: n = tanh(narg + b_nx)
  7. DVE/Pool: h' = n + z * (h - n)
Output is written hidden-major [128, 2048] per core and transposed
back on the host.
"""

import os

import numpy as np

N_NODES = 200000
D = 128          # input dim
H = 128          # hidden dim
B_TOTAL = 16384  # total walks
T = 20           # walk length
NCORES = 8
B = B_TOTAL // NCORES   # walks per core = 2048
NTILE = B // 128        # 16 walk tiles per step
CHUNK = 512             # batch chunk (one PSUM bank at fp32)
NCHUNK = B // CHUNK     # 4
HALF = B // 2           # elementwise granularity for h-update

_CACHE = {}


def _build_module(n_steps=T, batch=B, debug_taps=False):
    import concourse.bass as bass
    import concourse.mybir as mybir
    import concourse.tile as tile
    from concourse import bacc
    from concourse.masks import make_identity

    f32 = mybir.dt.float32
    f32r = mybir.dt.float32r
    i32 = mybir.dt.int32
    ntile = batch // 128
    nchunk = batch // CHUNK

    nc = bacc.Bacc(None, target_bir_lowering=False)

    x_d = nc.dram_tensor("x", [N_NODES, D], f32, kind="ExternalInput")
    idx_d = nc.dram_tensor("idx", [128, n_steps * ntile], i32, kind="ExternalInput")
    wc_d = nc.dram_tensor("wct", [128, 3 * H], f32r, kind="ExternalInput")
    wh_d = nc.dram_tensor("wht", [128, 3 * H], f32r, kind="ExternalInput")
    b_d = nc.dram_tensor("bias", [128, 4], f32, kind="ExternalInput")
    out_d = nc.dram_tensor("out", [128, batch], f32, kind="ExternalOutput")
    if debug_taps:
        tapG_d = nc.dram_tensor("tapG", [128, batch], f32, kind="ExternalOutput")
        tapxT_d = nc.dram_tensor("tapxT", [128, batch], f32, kind="ExternalOutput")
        tapr_d = nc.dram_tensor("tapr", [128, batch], f32, kind="ExternalOutput")
        tapn_d = nc.dram_tensor("tapn", [128, batch], f32, kind="ExternalOutput")
        taph0_d = nc.dram_tensor("taph0", [128, batch], f32, kind="ExternalOutput")

    Sig = mybir.ActivationFunctionType.Sigmoid
    Tanh = mybir.ActivationFunctionType.Tanh
    Alu = mybir.AluOpType

    with tile.TileContext(nc) as tc:
        with tc.tile_pool(name="cst", bufs=1) as cst, \
             tc.tile_pool(name="sb", bufs=2) as sb, \
             tc.tile_pool(name="ps", bufs=1, space="PSUM") as ps:

            wc = cst.tile([128, 3 * H], f32r, name="wc")
            nc.sync.dma_start(wc[:], wc_d[:])
            wh = cst.tile([128, 3 * H], f32r, name="wh")
            nc.sync.dma_start(wh[:], wh_d[:])
            bias = cst.tile([128, 4], f32, name="biast")
            nc.sync.dma_start(bias[:], b_d[:])
            idx0 = cst.tile([128, n_steps * ntile], i32, name="idxt")
            nc.sync.dma_start(idx0[:], idx_d[:])
            # Pool-engine touch of the index data: forces Q7-visible ordering
            # before any indirect-DMA descriptor generation (stale-read flake
            # seen otherwise on the first gather).
            idx = cst.tile([128, n_steps * ntile], i32, name="idxt2")
            nc.gpsimd.tensor_copy(idx[:], idx0[:])
            ident = cst.tile([128, 128], f32, name="ident")
            make_identity(nc, ident[:])
            scratch = cst.tile([128, 128], f32, name="scratch")
            nc.gpsimd.indirect_dma_start(
                out=scratch[:], out_offset=None, in_=x_d[:],
                in_offset=bass.IndirectOffsetOnAxis(ap=idx[:, 0:1], axis=0))

            b_r = bias[:, 0:1]
            b_z = bias[:, 1:2]
            b_hn = bias[:, 2:3]
            b_xn = bias[:, 3:4]

            h_prev = None
            for t in range(n_steps):
                # ---- gather this step's x rows ------------------------------
                G = sb.tile([128, batch], f32, tag="G", bufs=2, name=f"G{t}")
                for j in range(ntile):
                    nc.gpsimd.indirect_dma_start(
                        out=G[:, j * 128:(j + 1) * 128],
                        out_offset=None,
                        in_=x_d[:],
                        in_offset=bass.IndirectOffsetOnAxis(
                            ap=idx[:, t * ntile + j:t * ntile + j + 1], axis=0
                        ),
                    )

                # ---- transpose to feature-major -----------------------------
                xT = sb.tile([128, batch], f32r, tag="xT", bufs=2, name=f"xT{t}")
                for c in range(nchunk):
                    pT = ps.tile([128, CHUNK], f32, tag="pT", bufs=2, name=f"pT{t}_{c}")
                    for q in range(4):
                        j = 4 * c + q
                        nc.tensor.transpose(
                            out=pT[:, q * 128:(q + 1) * 128],
                            in_=G[:, j * 128:(j + 1) * 128],
                            identity=ident[:],
                        )
                    # PSUM -> SBUF move (DMA cannot touch PSUM); alternate
                    # engines to balance load
                    if c % 2 == 0:
                        nc.scalar.copy(xT[:, c * CHUNK:(c + 1) * CHUNK], pT[:])
                    else:
                        nc.vector.tensor_copy(xT[:, c * CHUNK:(c + 1) * CHUNK], pT[:])

                rbuf = sb.tile([128, batch], f32, tag="rbuf", bufs=2, name=f"r{t}")
                zbuf = sb.tile([128, batch], f32, tag="zbuf", bufs=2, name=f"z{t}")
                narg = sb.tile([128, batch], f32, tag="narg", bufs=2, name=f"na{t}")
                nbuf = sb.tile([128, batch], f32, tag="nbuf", bufs=2, name=f"n{t}")
                dbuf = sb.tile([128, batch], f32, tag="dbuf", bufs=2, name=f"d{t}")
                zd = sb.tile([128, batch], f32, tag="zd", bufs=2, name=f"zd{t}")
                h_new = sb.tile([128, batch], f32r, tag="h", bufs=2, name=f"h{t}")

                for c in range(nchunk):
                    S = slice(c * CHUNK, (c + 1) * CHUNK)
                    xc = xT[:, S]

                    p_r = ps.tile([128, CHUNK], f32, tag="p_r", bufs=2, name=f"pr{t}_{c}")
                    p_z = ps.tile([128, CHUNK], f32, tag="p_z", bufs=2, name=f"pz{t}_{c}")
                    p_nx = ps.tile([128, CHUNK], f32, tag="p_nx", bufs=1, name=f"px{t}_{c}")

                    last = t == 0
                    nc.tensor.matmul(out=p_r[:], lhsT=wc[:, 0:128],
                                     rhs=xc, start=True, stop=last)
                    nc.tensor.matmul(out=p_z[:], lhsT=wc[:, 128:256],
                                     rhs=xc, start=True, stop=last)
                    nc.tensor.matmul(out=p_nx[:], lhsT=wc[:, 256:384],
                                     rhs=xc, start=True, stop=True)
                    if t > 0:
                        hc = h_prev[:, S]
                        nc.tensor.matmul(out=p_r[:], lhsT=wh[:, 0:128],
                                         rhs=hc, start=False, stop=True)
                        nc.tensor.matmul(out=p_z[:], lhsT=wh[:, 128:256],
                                         rhs=hc, start=False, stop=True)
                        p_nh = ps.tile([128, CHUNK], f32, tag="p_nh", bufs=1,
                                       name=f"ph{t}_{c}")
                        nc.tensor.matmul(out=p_nh[:], lhsT=wh[:, 256:384],
                                         rhs=hc, start=True, stop=True)

                    # gates
                    nc.scalar.activation(out=rbuf[:, S], in_=p_r[:], func=Sig, bias=b_r)
                    nc.scalar.activation(out=zbuf[:, S], in_=p_z[:], func=Sig, bias=b_z)

                    rhn = sb.tile([128, CHUNK], f32, tag="rhn", bufs=2,
                                  name=f"rhn{t}_{c}")
                    if t > 0:
                        # rhn = (p_nh + b_hn) * r
                        nc.vector.scalar_tensor_tensor(
                            out=rhn[:], in0=p_nh[:], scalar=b_hn, in1=rbuf[:, S],
                            op0=Alu.add, op1=Alu.mult)
                    else:
                        # h == 0 -> rhn = b_hn * r
                        nc.vector.tensor_scalar(
                            out=rhn[:], in0=rbuf[:, S], scalar1=b_hn, scalar2=None,
                            op0=Alu.mult)
                    nc.vector.tensor_tensor(out=narg[:, S], in0=rhn[:], in1=p_nx[:],
                                            op=Alu.add)
                    nc.scalar.activation(out=nbuf[:, S], in_=narg[:, S], func=Tanh,
                                         bias=b_xn)

                # ---- h' = n + z * (h - n), two halves -----------------------
                for m in range(2):
                    M = slice(m * (batch // 2), (m + 1) * (batch // 2))
                    if t > 0:
                        nc.gpsimd.tensor_tensor(out=dbuf[:, M], in0=h_prev[:, M],
                                                in1=nbuf[:, M], op=Alu.subtract)
                    else:
                        nc.gpsimd.tensor_scalar(out=dbuf[:, M], in0=nbuf[:, M],
                                                scalar1=-1.0, scalar2=None,
                                                op0=Alu.mult)
                    nc.vector.tensor_tensor(out=zd[:, M], in0=zbuf[:, M],
                                            in1=dbuf[:, M], op=Alu.mult)
                    nc.vector.tensor_tensor(out=h_new[:, M], in0=nbuf[:, M],
                                            in1=zd[:, M], op=Alu.add)

                if debug_taps and t == 0:
                    nc.sync.dma_start(tapG_d[:], G[:])
                    nc.sync.dma_start(tapxT_d[:], xT[:].bitcast(f32))
                    nc.sync.dma_start(tapr_d[:], rbuf[:])
                    nc.sync.dma_start(tapn_d[:], nbuf[:])
                    nc.sync.dma_start(taph0_d[:], h_new[:].bitcast(f32))
                h_prev = h_new

            nc.sync.dma_start(out_d[:], h_prev[:].bitcast(f32))

    nc.compile()
    return nc


def _get_module():
    key = "mod"
    if key not in _CACHE:
        _CACHE[key] = _build_module()
    return _CACHE[key]


def _host_prep(x, walks, W_f, b_f, W_ih, W_hh, b_ih, b_hh):
    """Fold encoder into GRU input projection; pack per-core inputs."""
    x = np.ascontiguousarray(np.asarray(x, dtype=np.float32))
    walks = np.asarray(walks).astype(np.int32)
    W_f = np.asarray(W_f, dtype=np.float32)
    b_f = np.asarray(b_f, dtype=np.float32)
    W_ih = np.asarray(W_ih, dtype=np.float32)
    W_hh = np.asarray(W_hh, dtype=np.float32)
    b_ih = np.asarray(b_ih, dtype=np.float32)
    b_hh = np.asarray(b_hh, dtype=np.float32)

    W_c = (W_ih @ W_f).astype(np.float32)          # [3H, D]
    b_c = (W_ih @ b_f + b_ih).astype(np.float32)   # [3H]

    wct = np.ascontiguousarray(W_c.T)              # [D, 3H] = lhsT layout
    wht = np.ascontiguousarray(W_hh.T)             # [H, 3H]

    # bias columns: b_r, b_z (include recurrent parts), b_hn, b_xn
    bias = np.zeros((128, 4), dtype=np.float32)
    bias[:, 0] = b_c[0:128] + b_hh[0:128]
    bias[:, 1] = b_c[128:256] + b_hh[128:256]
    bias[:, 2] = b_hh[256:384]
    bias[:, 3] = b_c[256:384]

    in_maps = []
    for core in range(NCORES):
        w = walks[core * B:(core + 1) * B]                 # [B, T]
        # idx[p, t*NTILE + j] = walks[j*128 + p, t]
        idx = np.ascontiguousarray(
            w.reshape(NTILE, 128, T).transpose(1, 2, 0).reshape(128, T * NTILE)
        ).astype(np.int32)
        in_maps.append({
            "x": x,
            "idx": idx,
            "wct": wct,
            "wht": wht,
            "bias": bias,
        })
    return in_maps


def kernel(x, walks, W_f, b_f, W_ih, W_hh, b_ih, b_hh):
    from concourse.bass_utils import run_bass_kernel_spmd

    in_maps = _host_prep(x, walks, W_f, b_f, W_ih, W_hh, b_ih, b_hh)
    nc = _get_module()
    res = run_bass_kernel_spmd(nc, in_maps, core_ids=list(range(NCORES)))
    out = np.empty((B_TOTAL, H), dtype=np.float32)
    for core in range(NCORES):
        out[core * B:(core + 1) * B] = res.results[core]["out"].T
    return out


if __name__ == "__main__":
    rng = np.random.default_rng(0)
    ins = {
        "x": rng.standard_normal((N_NODES, D), dtype=np.float32),
        "walks": rng.integers(0, N_NODES, size=(B_TOTAL, T)).astype(np.int32),
        "W_f": rng.standard_normal((H, D), dtype=np.float32) / np.sqrt(D),
        "b_f": np.zeros(H, np.float32),
        "W_ih": rng.standard_normal((3 * H, H), dtype=np.float32) / np.sqrt(H),
        "W_hh": rng.standard_normal((3 * H, H), dtype=np.float32) / np.sqrt(H),
        "b_ih": np.zeros(3 * H, np.float32),
        "b_hh": np.zeros(3 * H, np.float32),
    }
    out = kernel(**ins)
    print(out.shape, out.dtype, np.abs(out).mean())


# revision 9
# speedup vs baseline: 1.1988x; 1.1988x over previous
"""Trainium2 Bass kernel for ExplicitRandomWalkEncoder.

Math (reference):
    x_encoded = x @ W_f.T + b_f                      # [N, H]
    feats     = x_encoded[walks]                     # [B, T, H]
    h_T       = GRU(feats)  (torch gate order r,z,n) # [B, H]

Key algebraic fold: the feature encoder commutes into the GRU input
projection, so the device never materializes x_encoded:
    gx = feats @ W_ih.T + b_ih
       = x[walks] @ (W_ih @ W_f).T + (W_ih @ b_f + b_ih)
The kernel gathers raw x rows (512B each) and applies the combined
input projection W_c = W_ih @ W_f.

Sharding: data-parallel over walks. Each of the 8 cores processes
2048 walks; x and all weights are replicated. No collectives.

Per-core per-step dataflow (hidden-major layout, hidden dim on
partitions, batch on free dim):
  1. indirect DMA gathers 2048 rows of x -> G [128 batch-part, 16*128]
  2. PE transposes each [128,128] walk-tile -> xT [128 feat, 2048]
  3. PE matmuls (float32r): per 512-batch chunk, psum_r/z = W_c_g @ xT
     accumulated with W_hh_g @ h; psum_nx, psum_nh separate
  4. ACT: r = sigmoid(psum_r + b_r), z likewise
  5. DVE: rhn = (psum_nh + b_hn) * r ; narg = rhn + psum_nx
  6. ACT: n = tanh(narg + b_nx)
  7. DVE/Pool: h' = n + z * (h - n)
Output is written hidden-major [128, 2048] per core and transposed
back on the host.
"""

import os

import numpy as np

N_NODES = 200000
D = 128          # input dim
H = 128          # hidden dim
B_TOTAL = 16384  # total walks
T = 20           # walk length
NCORES = 8
B = B_TOTAL // NCORES   # walks per core = 2048
NTILE = B // 128        # 16 walk tiles per step
CHUNK = 512             # batch chunk (one PSUM bank at fp32)
NCHUNK = B // CHUNK     # 4
HALF = B // 2           # elementwise granularity for h-update

_CACHE = {}


def _build_module(n_steps=T, batch=B, debug_taps=False):
    import concourse.bass as bass
    import concourse.mybir as mybir
    import concourse.tile as tile
    from concourse import bacc
    from concourse.masks import make_identity

    f32 = mybir.dt.float32
    f32r = mybir.dt.float32r
    i32 = mybir.dt.int32
    ntile = batch // 128
    nchunk = batch // CHUNK

    nc = bacc.Bacc(None, target_bir_lowering=False)

    x_d = nc.dram_tensor("x", [N_NODES, D], f32, kind="ExternalInput")
    idx_d = nc.dram_tensor("idx", [128, n_steps * ntile], i32, kind="ExternalInput")
    wc_d = nc.dram_tensor("wct", [128, 3 * H], f32r, kind="ExternalInput")
    wh_d = nc.dram_tensor("wht", [128, 3 * H], f32r, kind="ExternalInput")
    b_d = nc.dram_tensor("bias", [128, 4], f32, kind="ExternalInput")
    out_d = nc.dram_tensor("out", [128, batch], f32, kind="ExternalOutput")
    if debug_taps:
        tapG_d = nc.dram_tensor("tapG", [128, batch], f32, kind="ExternalOutput")
        tapxT_d = nc.dram_tensor("tapxT", [128, batch], f32, kind="ExternalOutput")
        tapr_d = nc.dram_tensor("tapr", [128, batch], f32, kind="ExternalOutput")
        tapn_d = nc.dram_tensor("tapn", [128, batch], f32, kind="ExternalOutput")
        taph0_d = nc.dram_tensor("taph0", [128, batch], f32, kind="ExternalOutput")

    Sig = mybir.ActivationFunctionType.Sigmoid
    Tanh = mybir.ActivationFunctionType.Tanh
    Alu = mybir.AluOpType

    with tile.TileContext(nc) as tc:
        with tc.tile_pool(name="cst", bufs=1) as cst, \
             tc.tile_pool(name="sb", bufs=2) as sb, \
             tc.tile_pool(name="ps", bufs=1, space="PSUM") as ps:

            wc = cst.tile([128, 3 * H], f32r, name="wc")
            nc.sync.dma_start(wc[:], wc_d[:])
            wh = cst.tile([128, 3 * H], f32r, name="wh")
            nc.sync.dma_start(wh[:], wh_d[:])
            bias = cst.tile([128, 4], f32, name="biast")
            nc.sync.dma_start(bias[:], b_d[:])
            idx0 = cst.tile([128, n_steps * ntile], i32, name="idxt")
            nc.sync.dma_start(idx0[:], idx_d[:])
            # Pool-engine touch of the index data: forces Q7-visible ordering
            # before any indirect-DMA descriptor generation (stale-read flake
            # seen otherwise on the first gather).
            idx = cst.tile([128, n_steps * ntile], i32, name="idxt2")
            nc.gpsimd.tensor_copy(idx[:], idx0[:])
            ident = cst.tile([128, 128], f32, name="ident")
            make_identity(nc, ident[:])
            scratch = cst.tile([128, 128], f32, name="scratch")
            nc.gpsimd.indirect_dma_start(
                out=scratch[:], out_offset=None, in_=x_d[:],
                in_offset=bass.IndirectOffsetOnAxis(ap=idx[:, 0:1], axis=0))

            b_r = bias[:, 0:1]
            b_z = bias[:, 1:2]
            b_hn = bias[:, 2:3]
            b_xn = bias[:, 3:4]

            h_prev = None
            for t in range(n_steps):
                # ---- gather this step's x rows ------------------------------
                G = sb.tile([128, batch], f32, tag="G", bufs=3, name=f"G{t}")
                for j in range(ntile):
                    nc.gpsimd.indirect_dma_start(
                        out=G[:, j * 128:(j + 1) * 128],
                        out_offset=None,
                        in_=x_d[:],
                        in_offset=bass.IndirectOffsetOnAxis(
                            ap=idx[:, t * ntile + j:t * ntile + j + 1], axis=0
                        ),
                    )

                # ---- transpose to feature-major -----------------------------
                xT = sb.tile([128, batch], f32r, tag="xT", bufs=3, name=f"xT{t}")
                for c in range(nchunk):
                    pT = ps.tile([128, CHUNK], f32, tag="pT", bufs=2, name=f"pT{t}_{c}")
                    for q in range(4):
                        j = 4 * c + q
                        nc.tensor.transpose(
                            out=pT[:, q * 128:(q + 1) * 128],
                            in_=G[:, j * 128:(j + 1) * 128],
                            identity=ident[:],
                        )
                    # PSUM -> SBUF move (DMA cannot touch PSUM); alternate
                    # engines to balance load
                    nc.scalar.copy(xT[:, c * CHUNK:(c + 1) * CHUNK], pT[:])

                rbuf = sb.tile([128, batch], f32, tag="rbuf", bufs=2, name=f"r{t}")
                zbuf = sb.tile([128, batch], f32, tag="zbuf", bufs=2, name=f"z{t}")
                narg = sb.tile([128, batch], f32, tag="narg", bufs=2, name=f"na{t}")
                nbuf = sb.tile([128, batch], f32, tag="nbuf", bufs=2, name=f"n{t}")
                dbuf = sb.tile([128, batch], f32, tag="dbuf", bufs=2, name=f"d{t}")
                zd = sb.tile([128, batch], f32, tag="zd", bufs=2, name=f"zd{t}")
                h_new = sb.tile([128, batch], f32r, tag="h", bufs=2, name=f"h{t}")

                for c in range(nchunk):
                    S = slice(c * CHUNK, (c + 1) * CHUNK)
                    xc = xT[:, S]

                    p_r = ps.tile([128, CHUNK], f32, tag="p_r", bufs=2, name=f"pr{t}_{c}")
                    p_z = ps.tile([128, CHUNK], f32, tag="p_z", bufs=2, name=f"pz{t}_{c}")
                    p_nx = ps.tile([128, CHUNK], f32, tag="p_nx", bufs=1, name=f"px{t}_{c}")

                    last = t == 0
                    nc.tensor.matmul(out=p_r[:], lhsT=wc[:, 0:128],
                                     rhs=xc, start=True, stop=last)
                    nc.tensor.matmul(out=p_z[:], lhsT=wc[:, 128:256],
                                     rhs=xc, start=True, stop=last)
                    nc.tensor.matmul(out=p_nx[:], lhsT=wc[:, 256:384],
                                     rhs=xc, start=True, stop=True)
                    if t > 0:
                        hc = h_prev[:, S]
                        nc.tensor.matmul(out=p_r[:], lhsT=wh[:, 0:128],
                                         rhs=hc, start=False, stop=True)
                        nc.tensor.matmul(out=p_z[:], lhsT=wh[:, 128:256],
                                         rhs=hc, start=False, stop=True)
                        p_nh = ps.tile([128, CHUNK], f32, tag="p_nh", bufs=1,
                                       name=f"ph{t}_{c}")
                        nc.tensor.matmul(out=p_nh[:], lhsT=wh[:, 256:384],
                                         rhs=hc, start=True, stop=True)

                    # gates
                    nc.scalar.activation(out=rbuf[:, S], in_=p_r[:], func=Sig, bias=b_r)
                    nc.scalar.activation(out=zbuf[:, S], in_=p_z[:], func=Sig, bias=b_z)

                    rhn = sb.tile([128, CHUNK], f32, tag="rhn", bufs=2,
                                  name=f"rhn{t}_{c}")
                    if t > 0:
                        # rhn = (p_nh + b_hn) * r
                        nc.vector.scalar_tensor_tensor(
                            out=rhn[:], in0=p_nh[:], scalar=b_hn, in1=rbuf[:, S],
                            op0=Alu.add, op1=Alu.mult)
                    else:
                        # h == 0 -> rhn = b_hn * r
                        nc.vector.tensor_scalar(
                            out=rhn[:], in0=rbuf[:, S], scalar1=b_hn, scalar2=None,
                            op0=Alu.mult)
                    nc.vector.tensor_tensor(out=narg[:, S], in0=rhn[:], in1=p_nx[:],
                                            op=Alu.add)
                    nc.scalar.activation(out=nbuf[:, S], in_=narg[:, S], func=Tanh,
                                         bias=b_xn)

                # ---- h' = n + z * (h - n), two halves -----------------------
                for m in range(2):
                    M = slice(m * (batch // 2), (m + 1) * (batch // 2))
                    if t > 0:
                        nc.vector.tensor_tensor(out=dbuf[:, M], in0=h_prev[:, M],
                                                in1=nbuf[:, M], op=Alu.subtract)
                    else:
                        nc.vector.tensor_scalar(out=dbuf[:, M], in0=nbuf[:, M],
                                                scalar1=-1.0, scalar2=None,
                                                op0=Alu.mult)
                    nc.vector.tensor_tensor(out=zd[:, M], in0=zbuf[:, M],
                                            in1=dbuf[:, M], op=Alu.mult)
                    nc.vector.tensor_tensor(out=h_new[:, M], in0=nbuf[:, M],
                                            in1=zd[:, M], op=Alu.add)

                if debug_taps and t == 0:
                    nc.sync.dma_start(tapG_d[:], G[:])
                    nc.sync.dma_start(tapxT_d[:], xT[:].bitcast(f32))
                    nc.sync.dma_start(tapr_d[:], rbuf[:])
                    nc.sync.dma_start(tapn_d[:], nbuf[:])
                    nc.sync.dma_start(taph0_d[:], h_new[:].bitcast(f32))
                h_prev = h_new

            nc.sync.dma_start(out_d[:], h_prev[:].bitcast(f32))

    nc.compile()
    return nc


def _get_module():
    key = "mod"
    if key not in _CACHE:
        _CACHE[key] = _build_module()
    return _CACHE[key]


def _host_prep(x, walks, W_f, b_f, W_ih, W_hh, b_ih, b_hh):
    """Fold encoder into GRU input projection; pack per-core inputs."""
    x = np.ascontiguousarray(np.asarray(x, dtype=np.float32))
    walks = np.asarray(walks).astype(np.int32)
    W_f = np.asarray(W_f, dtype=np.float32)
    b_f = np.asarray(b_f, dtype=np.float32)
    W_ih = np.asarray(W_ih, dtype=np.float32)
    W_hh = np.asarray(W_hh, dtype=np.float32)
    b_ih = np.asarray(b_ih, dtype=np.float32)
    b_hh = np.asarray(b_hh, dtype=np.float32)

    W_c = (W_ih @ W_f).astype(np.float32)          # [3H, D]
    b_c = (W_ih @ b_f + b_ih).astype(np.float32)   # [3H]

    wct = np.ascontiguousarray(W_c.T)              # [D, 3H] = lhsT layout
    wht = np.ascontiguousarray(W_hh.T)             # [H, 3H]

    # bias columns: b_r, b_z (include recurrent parts), b_hn, b_xn
    bias = np.zeros((128, 4), dtype=np.float32)
    bias[:, 0] = b_c[0:128] + b_hh[0:128]
    bias[:, 1] = b_c[128:256] + b_hh[128:256]
    bias[:, 2] = b_hh[256:384]
    bias[:, 3] = b_c[256:384]

    in_maps = []
    for core in range(NCORES):
        w = walks[core * B:(core + 1) * B]                 # [B, T]
        # idx[p, t*NTILE + j] = walks[j*128 + p, t]
        idx = np.ascontiguousarray(
            w.reshape(NTILE, 128, T).transpose(1, 2, 0).reshape(128, T * NTILE)
        ).astype(np.int32)
        in_maps.append({
            "x": x,
            "idx": idx,
            "wct": wct,
            "wht": wht,
            "bias": bias,
        })
    return in_maps


def kernel(x, walks, W_f, b_f, W_ih, W_hh, b_ih, b_hh):
    from concourse.bass_utils import run_bass_kernel_spmd

    in_maps = _host_prep(x, walks, W_f, b_f, W_ih, W_hh, b_ih, b_hh)
    nc = _get_module()
    res = run_bass_kernel_spmd(nc, in_maps, core_ids=list(range(NCORES)))
    out = np.empty((B_TOTAL, H), dtype=np.float32)
    for core in range(NCORES):
        out[core * B:(core + 1) * B] = res.results[core]["out"].T
    return out


if __name__ == "__main__":
    rng = np.random.default_rng(0)
    ins = {
        "x": rng.standard_normal((N_NODES, D), dtype=np.float32),
        "walks": rng.integers(0, N_NODES, size=(B_TOTAL, T)).astype(np.int32),
        "W_f": rng.standard_normal((H, D), dtype=np.float32) / np.sqrt(D),
        "b_f": np.zeros(H, np.float32),
        "W_ih": rng.standard_normal((3 * H, H), dtype=np.float32) / np.sqrt(H),
        "W_hh": rng.standard_normal((3 * H, H), dtype=np.float32) / np.sqrt(H),
        "b_ih": np.zeros(3 * H, np.float32),
        "b_hh": np.zeros(3 * H, np.float32),
    }
    out = kernel(**ins)
    print(out.shape, out.dtype, np.abs(out).mean())


# revision 11
# speedup vs baseline: 1.2795x; 1.0673x over previous
"""Trainium2 Bass kernel for ExplicitRandomWalkEncoder.

Math (reference):
    x_encoded = x @ W_f.T + b_f                      # [N, H]
    feats     = x_encoded[walks]                     # [B, T, H]
    h_T       = GRU(feats)  (torch gate order r,z,n) # [B, H]

Key algebraic fold: the feature encoder commutes into the GRU input
projection, so the device never materializes x_encoded:
    gx = feats @ W_ih.T + b_ih
       = x[walks] @ (W_ih @ W_f).T + (W_ih @ b_f + b_ih)
The kernel gathers raw x rows (512B each) and applies the combined
input projection W_c = W_ih @ W_f.

Sharding: data-parallel over walks. Each of the 8 cores processes
2048 walks; x and all weights are replicated. No collectives.

Per-core per-step dataflow (hidden-major layout, hidden dim on
partitions, batch on free dim):
  1. indirect DMA gathers 2048 rows of x -> G [128 batch-part, 16*128]
  2. PE transposes each [128,128] walk-tile -> xT [128 feat, 2048]
  3. PE matmuls (float32r): per 512-batch chunk, psum_r/z = W_c_g @ xT
     accumulated with W_hh_g @ h; psum_nx, psum_nh separate
  4. ACT: r = sigmoid(psum_r + b_r), z likewise
  5. DVE: rhn = (psum_nh + b_hn) * r ; narg = rhn + psum_nx
  6. ACT: n = tanh(narg + b_nx)
  7. DVE/Pool: h' = n + z * (h - n)
Output is written hidden-major [128, 2048] per core and transposed
back on the host.
"""

import os

import numpy as np

N_NODES = 200000
D = 128          # input dim
H = 128          # hidden dim
B_TOTAL = 16384  # total walks
T = 20           # walk length
NCORES = 8
B = B_TOTAL // NCORES   # walks per core = 2048
NTILE = B // 128        # 16 walk tiles per step
CHUNK = 512             # batch chunk (one PSUM bank at fp32)
NCHUNK = B // CHUNK     # 4
HALF = B // 2           # elementwise granularity for h-update

_CACHE = {}


def _build_module(n_steps=T, batch=B, debug_taps=False, rz_same_bias=True):
    import concourse.bass as bass
    import concourse.mybir as mybir
    import concourse.tile as tile
    from concourse import bacc
    from concourse.masks import make_identity

    f32 = mybir.dt.float32
    f32r = mybir.dt.float32r
    i32 = mybir.dt.int32
    ntile = batch // 128
    nchunk = batch // CHUNK

    nc = bacc.Bacc(None, target_bir_lowering=False)

    x_d = nc.dram_tensor("x", [N_NODES, D], f32, kind="ExternalInput")
    idx_d = nc.dram_tensor("idx", [128, n_steps * ntile], i32, kind="ExternalInput")
    wc_d = nc.dram_tensor("wct", [128, 3 * H], f32r, kind="ExternalInput")
    wh_d = nc.dram_tensor("wht", [128, 3 * H], f32r, kind="ExternalInput")
    b_d = nc.dram_tensor("bias", [128, 4], f32, kind="ExternalInput")
    out_d = nc.dram_tensor("out", [128, batch], f32, kind="ExternalOutput")
    if debug_taps:
        tapG_d = nc.dram_tensor("tapG", [128, batch], f32, kind="ExternalOutput")
        tapxT_d = nc.dram_tensor("tapxT", [128, batch], f32, kind="ExternalOutput")
        tapr_d = nc.dram_tensor("tapr", [128, batch], f32, kind="ExternalOutput")
        tapn_d = nc.dram_tensor("tapn", [128, batch], f32, kind="ExternalOutput")
        taph0_d = nc.dram_tensor("taph0", [128, batch], f32, kind="ExternalOutput")

    Sig = mybir.ActivationFunctionType.Sigmoid
    Tanh = mybir.ActivationFunctionType.Tanh
    Alu = mybir.AluOpType

    with tile.TileContext(nc) as tc:
        with tc.tile_pool(name="cst", bufs=1) as cst, \
             tc.tile_pool(name="sb", bufs=2) as sb, \
             tc.tile_pool(name="ps", bufs=1, space="PSUM") as ps:

            wc = cst.tile([128, 3 * H], f32r, name="wc")
            nc.sync.dma_start(wc[:], wc_d[:])
            wh = cst.tile([128, 3 * H], f32r, name="wh")
            nc.sync.dma_start(wh[:], wh_d[:])
            bias = cst.tile([128, 4], f32, name="biast")
            nc.sync.dma_start(bias[:], b_d[:])
            idx0 = cst.tile([128, n_steps * ntile], i32, name="idxt")
            nc.sync.dma_start(idx0[:], idx_d[:])
            # Pool-engine touch of the index data: forces Q7-visible ordering
            # before any indirect-DMA descriptor generation (stale-read flake
            # seen otherwise on the first gather).
            idx = cst.tile([128, n_steps * ntile], i32, name="idxt2")
            nc.gpsimd.tensor_copy(idx[:], idx0[:])
            ident = cst.tile([128, 128], f32, name="ident")
            make_identity(nc, ident[:])
            scratch = cst.tile([128, 128], f32, name="scratch")
            nc.gpsimd.indirect_dma_start(
                out=scratch[:], out_offset=None, in_=x_d[:],
                in_offset=bass.IndirectOffsetOnAxis(ap=idx[:, 0:1], axis=0))

            b_r = bias[:, 0:1]
            b_z = bias[:, 1:2]
            b_hn = bias[:, 2:3]
            b_xn = bias[:, 3:4]

            h_prev = None
            for t in range(n_steps):
                # ---- gather this step's x rows (one row per partition/instr)
                G = sb.tile([128, batch], f32, tag="G", bufs=3, name=f"G{t}")
                for j in range(ntile):
                    nc.gpsimd.indirect_dma_start(
                        out=G[:, j * 128:(j + 1) * 128],
                        out_offset=None,
                        in_=x_d[:],
                        in_offset=bass.IndirectOffsetOnAxis(
                            ap=idx[:, t * ntile + j:t * ntile + j + 1], axis=0
                        ),
                    )

                # ---- transpose to feature-major: 8 transposes per big tile
                xT = sb.tile([128, batch], f32r, tag="xT", bufs=3, name=f"xT{t}")
                for half in range(2):
                    pT = ps.tile([128, 1024], f32, tag="big", bufs=4,
                                 name=f"pT{t}_{half}")
                    for q in range(8):
                        j = 8 * half + q
                        nc.tensor.transpose(
                            out=pT[:, q * 128:(q + 1) * 128],
                            in_=G[:, j * 128:(j + 1) * 128],
                            identity=ident[:],
                        )
                    nc.scalar.copy(xT[:, half * 1024:(half + 1) * 1024], pT[:])

                rzbuf = sb.tile([128, 2 * batch], f32, tag="rzbuf", bufs=2,
                                name=f"rz{t}")
                narg = sb.tile([128, batch], f32, tag="narg", bufs=2, name=f"na{t}")
                nbuf = sb.tile([128, batch], f32, tag="nbuf", bufs=2, name=f"n{t}")
                dbuf = sb.tile([128, batch], f32, tag="dbuf", bufs=2, name=f"d{t}")
                zd = sb.tile([128, batch], f32, tag="zd", bufs=2, name=f"zd{t}")
                h_new = sb.tile([128, batch], f32r, tag="h", bufs=2, name=f"h{t}")

                rz_tiles = {}
                nxh_tiles = {}
                # ---- gate matmuls, chunk-pair-outer for LDW amortization
                for pair in range(nchunk // 2):
                    cs = (2 * pair, 2 * pair + 1)
                    for c in cs:
                        rz_tiles[c] = ps.tile([128, 1024], f32, tag="big", bufs=4,
                                              name=f"rz{t}_{c}")
                        nxh_tiles[c] = ps.tile([128, 1024], f32, tag="big", bufs=4,
                                               name=f"nxh{t}_{c}")
                    last = t == 0
                    for c in cs:
                        nc.tensor.matmul(out=rz_tiles[c][:, 0:512],
                                         lhsT=wc[:, 0:128],
                                         rhs=xT[:, c * CHUNK:(c + 1) * CHUNK],
                                         start=True, stop=last)
                    for c in cs:
                        nc.tensor.matmul(out=rz_tiles[c][:, 512:1024],
                                         lhsT=wc[:, 128:256],
                                         rhs=xT[:, c * CHUNK:(c + 1) * CHUNK],
                                         start=True, stop=last)
                    for c in cs:
                        nc.tensor.matmul(out=nxh_tiles[c][:, 0:512],
                                         lhsT=wc[:, 256:384],
                                         rhs=xT[:, c * CHUNK:(c + 1) * CHUNK],
                                         start=True, stop=True)
                    if t > 0:
                        for c in cs:
                            nc.tensor.matmul(out=rz_tiles[c][:, 0:512],
                                             lhsT=wh[:, 0:128],
                                             rhs=h_prev[:, c * CHUNK:(c + 1) * CHUNK],
                                             start=False, stop=True)
                        for c in cs:
                            nc.tensor.matmul(out=rz_tiles[c][:, 512:1024],
                                             lhsT=wh[:, 128:256],
                                             rhs=h_prev[:, c * CHUNK:(c + 1) * CHUNK],
                                             start=False, stop=True)
                        for c in cs:
                            nc.tensor.matmul(out=nxh_tiles[c][:, 512:1024],
                                             lhsT=wh[:, 256:384],
                                             rhs=h_prev[:, c * CHUNK:(c + 1) * CHUNK],
                                             start=True, stop=True)

                    # ---- per-chunk gates + h update
                    for c in cs:
                        S = slice(c * CHUNK, (c + 1) * CHUNK)
                        rz_ps = rz_tiles[c]
                        nxh_ps = nxh_tiles[c]
                        if rz_same_bias:
                            out_ap = rzbuf[:].rearrange(
                                "p (g b) -> p g b", g=2)[:, :, S]
                            nc.scalar.activation(out=out_ap, in_=rz_ps[:],
                                                 func=Sig, bias=b_r)
                        else:
                            nc.scalar.activation(out=rzbuf[:, S], in_=rz_ps[:, 0:512],
                                                 func=Sig, bias=b_r)
                            nc.scalar.activation(out=rzbuf[:, batch + c * CHUNK:
                                                           batch + (c + 1) * CHUNK],
                                                 in_=rz_ps[:, 512:1024],
                                                 func=Sig, bias=b_z)
                        r_ap = rzbuf[:, S]
                        z_ap = rzbuf[:, batch + c * CHUNK:batch + (c + 1) * CHUNK]

                        rhn = sb.tile([128, CHUNK], f32, tag="rhn", bufs=2,
                                      name=f"rhn{t}_{c}")
                        if t > 0:
                            nc.vector.scalar_tensor_tensor(
                                out=rhn[:], in0=nxh_ps[:, 512:1024], scalar=b_hn,
                                in1=r_ap, op0=Alu.add, op1=Alu.mult)
                        else:
                            nc.vector.tensor_scalar(
                                out=rhn[:], in0=r_ap, scalar1=b_hn, scalar2=None,
                                op0=Alu.mult)
                        nc.vector.tensor_tensor(out=narg[:, S], in0=rhn[:],
                                                in1=nxh_ps[:, 0:512], op=Alu.add)
                        nc.scalar.activation(out=nbuf[:, S], in_=narg[:, S],
                                             func=Tanh, bias=b_xn)
                        if t > 0:
                            nc.vector.tensor_tensor(out=dbuf[:, S], in0=h_prev[:, S],
                                                    in1=nbuf[:, S], op=Alu.subtract)
                        else:
                            nc.vector.tensor_scalar(out=dbuf[:, S], in0=nbuf[:, S],
                                                    scalar1=-1.0, scalar2=None,
                                                    op0=Alu.mult)
                        nc.vector.tensor_tensor(out=zd[:, S], in0=z_ap,
                                                in1=dbuf[:, S], op=Alu.mult)
                        nc.vector.tensor_tensor(out=h_new[:, S], in0=nbuf[:, S],
                                                in1=zd[:, S], op=Alu.add)

                if debug_taps and t == 0:
                    nc.sync.dma_start(tapG_d[:], G[:])
                    nc.sync.dma_start(tapxT_d[:], xT[:].bitcast(f32))
                    nc.sync.dma_start(tapr_d[:], rzbuf[:, 0:batch])
                    nc.sync.dma_start(tapn_d[:], nbuf[:])
                    nc.sync.dma_start(taph0_d[:], h_new[:].bitcast(f32))
                h_prev = h_new

            nc.sync.dma_start(out_d[:], h_prev[:].bitcast(f32))

    nc.compile()
    return nc


def _get_module(rz_same_bias=True):
    key = ("mod", rz_same_bias)
    if key not in _CACHE:
        _CACHE[key] = _build_module(rz_same_bias=rz_same_bias)
    return _CACHE[key]


def _host_prep(x, walks, W_f, b_f, W_ih, W_hh, b_ih, b_hh):
    """Fold encoder into GRU input projection; pack per-core inputs."""
    x = np.ascontiguousarray(np.asarray(x, dtype=np.float32))
    walks = np.asarray(walks).astype(np.int32)
    W_f = np.asarray(W_f, dtype=np.float32)
    b_f = np.asarray(b_f, dtype=np.float32)
    W_ih = np.asarray(W_ih, dtype=np.float32)
    W_hh = np.asarray(W_hh, dtype=np.float32)
    b_ih = np.asarray(b_ih, dtype=np.float32)
    b_hh = np.asarray(b_hh, dtype=np.float32)

    W_c = (W_ih @ W_f).astype(np.float32)          # [3H, D]
    b_c = (W_ih @ b_f + b_ih).astype(np.float32)   # [3H]

    wct = np.ascontiguousarray(W_c.T)              # [D, 3H] = lhsT layout
    wht = np.ascontiguousarray(W_hh.T)             # [H, 3H]

    # bias columns: b_r, b_z (include recurrent parts), b_hn, b_xn
    bias = np.zeros((128, 4), dtype=np.float32)
    bias[:, 0] = b_c[0:128] + b_hh[0:128]
    bias[:, 1] = b_c[128:256] + b_hh[128:256]
    bias[:, 2] = b_hh[256:384]
    bias[:, 3] = b_c[256:384]

    in_maps = []
    for core in range(NCORES):
        w = walks[core * B:(core + 1) * B]                 # [B, T]
        # idx[p, t*NTILE + j] = walks[j*128 + p, t]
        idx = np.ascontiguousarray(
            w.reshape(NTILE, 128, T).transpose(1, 2, 0).reshape(128, T * NTILE)
        ).astype(np.int32)
        in_maps.append({
            "x": x,
            "idx": idx,
            "wct": wct,
            "wht": wht,
            "bias": bias,
        })
    return in_maps


def kernel(x, walks, W_f, b_f, W_ih, W_hh, b_ih, b_hh):
    from concourse.bass_utils import run_bass_kernel_spmd

    in_maps = _host_prep(x, walks, W_f, b_f, W_ih, W_hh, b_ih, b_hh)
    bias = in_maps[0]["bias"]
    nc = _get_module(rz_same_bias=bool(np.allclose(bias[:, 0], bias[:, 1])))
    res = run_bass_kernel_spmd(nc, in_maps, core_ids=list(range(NCORES)))
    out = np.empty((B_TOTAL, H), dtype=np.float32)
    for core in range(NCORES):
        out[core * B:(core + 1) * B] = res.results[core]["out"].T
    return out


if __name__ == "__main__":
    rng = np.random.default_rng(0)
    ins = {
        "x": rng.standard_normal((N_NODES, D), dtype=np.float32),
        "walks": rng.integers(0, N_NODES, size=(B_TOTAL, T)).astype(np.int32),
        "W_f": rng.standard_normal((H, D), dtype=np.float32) / np.sqrt(D),
        "b_f": np.zeros(H, np.float32),
        "W_ih": rng.standard_normal((3 * H, H), dtype=np.float32) / np.sqrt(H),
        "W_hh": rng.standard_normal((3 * H, H), dtype=np.float32) / np.sqrt(H),
        "b_ih": np.zeros(3 * H, np.float32),
        "b_hh": np.zeros(3 * H, np.float32),
    }
    out = kernel(**ins)
    print(out.shape, out.dtype, np.abs(out).mean())


# revision 12
# speedup vs baseline: 1.2820x; 1.0019x over previous
"""Trainium2 Bass kernel for ExplicitRandomWalkEncoder.

Math (reference):
    x_encoded = x @ W_f.T + b_f                      # [N, H]
    feats     = x_encoded[walks]                     # [B, T, H]
    h_T       = GRU(feats)  (torch gate order r,z,n) # [B, H]

Key algebraic fold: the feature encoder commutes into the GRU input
projection, so the device never materializes x_encoded:
    gx = feats @ W_ih.T + b_ih
       = x[walks] @ (W_ih @ W_f).T + (W_ih @ b_f + b_ih)
The kernel gathers raw x rows (512B each) and applies the combined
input projection W_c = W_ih @ W_f.

Sharding: data-parallel over walks. Each of the 8 cores processes
2048 walks; x and all weights are replicated. No collectives.

Per-core per-step dataflow (hidden-major layout, hidden dim on
partitions, batch on free dim):
  1. indirect DMA gathers 2048 rows of x -> G [128 batch-part, 16*128]
  2. PE transposes each [128,128] walk-tile -> xT [128 feat, 2048]
  3. PE matmuls (float32r): per 512-batch chunk, psum_r/z = W_c_g @ xT
     accumulated with W_hh_g @ h; psum_nx, psum_nh separate
  4. ACT: r = sigmoid(psum_r + b_r), z likewise
  5. DVE: rhn = (psum_nh + b_hn) * r ; narg = rhn + psum_nx
  6. ACT: n = tanh(narg + b_nx)
  7. DVE/Pool: h' = n + z * (h - n)
Output is written hidden-major [128, 2048] per core and transposed
back on the host.
"""

import os

import numpy as np

N_NODES = 200000
D = 128          # input dim
H = 128          # hidden dim
B_TOTAL = 16384  # total walks
T = 20           # walk length
NCORES = 8
B = B_TOTAL // NCORES   # walks per core = 2048
NTILE = B // 128        # 16 walk tiles per step
CHUNK = 512             # batch chunk (one PSUM bank at fp32)
NCHUNK = B // CHUNK     # 4
HALF = B // 2           # elementwise granularity for h-update

_CACHE = {}


def _build_module(n_steps=T, batch=B, debug_taps=False, rz_same_bias=True):
    import concourse.bass as bass
    import concourse.mybir as mybir
    import concourse.tile as tile
    from concourse import bacc
    from concourse.masks import make_identity

    f32 = mybir.dt.float32
    f32r = mybir.dt.float32r
    i32 = mybir.dt.int32
    ntile = batch // 128
    nchunk = batch // CHUNK

    nc = bacc.Bacc(None, target_bir_lowering=False, num_swdge_queues=4)

    x_d = nc.dram_tensor("x", [N_NODES, D], f32, kind="ExternalInput")
    idx_d = nc.dram_tensor("idx", [128, n_steps * ntile], i32, kind="ExternalInput")
    wc_d = nc.dram_tensor("wct", [128, 3 * H], f32r, kind="ExternalInput")
    wh_d = nc.dram_tensor("wht", [128, 3 * H], f32r, kind="ExternalInput")
    b_d = nc.dram_tensor("bias", [128, 4], f32, kind="ExternalInput")
    out_d = nc.dram_tensor("out", [128, batch], f32, kind="ExternalOutput")
    if debug_taps:
        tapG_d = nc.dram_tensor("tapG", [128, batch], f32, kind="ExternalOutput")
        tapxT_d = nc.dram_tensor("tapxT", [128, batch], f32, kind="ExternalOutput")
        tapr_d = nc.dram_tensor("tapr", [128, batch], f32, kind="ExternalOutput")
        tapn_d = nc.dram_tensor("tapn", [128, batch], f32, kind="ExternalOutput")
        taph0_d = nc.dram_tensor("taph0", [128, batch], f32, kind="ExternalOutput")

    Sig = mybir.ActivationFunctionType.Sigmoid
    Tanh = mybir.ActivationFunctionType.Tanh
    Alu = mybir.AluOpType

    with tile.TileContext(nc) as tc:
        with tc.tile_pool(name="cst", bufs=1) as cst, \
             tc.tile_pool(name="sb", bufs=2) as sb, \
             tc.tile_pool(name="ps", bufs=1, space="PSUM") as ps:

            wc = cst.tile([128, 3 * H], f32r, name="wc")
            nc.sync.dma_start(wc[:], wc_d[:])
            wh = cst.tile([128, 3 * H], f32r, name="wh")
            nc.sync.dma_start(wh[:], wh_d[:])
            bias = cst.tile([128, 4], f32, name="biast")
            nc.sync.dma_start(bias[:], b_d[:])
            idx0 = cst.tile([128, n_steps * ntile], i32, name="idxt")
            nc.sync.dma_start(idx0[:], idx_d[:])
            # Pool-engine touch of the index data: forces Q7-visible ordering
            # before any indirect-DMA descriptor generation (stale-read flake
            # seen otherwise on the first gather).
            idx = cst.tile([128, n_steps * ntile], i32, name="idxt2")
            nc.gpsimd.tensor_copy(idx[:], idx0[:])
            ident = cst.tile([128, 128], f32, name="ident")
            make_identity(nc, ident[:])
            scratch = cst.tile([128, 128], f32, name="scratch")
            nc.gpsimd.indirect_dma_start(
                out=scratch[:], out_offset=None, in_=x_d[:],
                in_offset=bass.IndirectOffsetOnAxis(ap=idx[:, 0:1], axis=0))

            b_r = bias[:, 0:1]
            b_z = bias[:, 1:2]
            b_hn = bias[:, 2:3]
            b_xn = bias[:, 3:4]

            h_prev = None
            for t in range(n_steps):
                # ---- gather this step's x rows (one row per partition/instr)
                G = sb.tile([128, batch], f32, tag="G", bufs=3, name=f"G{t}")
                for j in range(ntile):
                    gi = nc.gpsimd.indirect_dma_start(
                        out=G[:, j * 128:(j + 1) * 128],
                        out_offset=None,
                        in_=x_d[:],
                        in_offset=bass.IndirectOffsetOnAxis(
                            ap=idx[:, t * ntile + j:t * ntile + j + 1], axis=0
                        ),
                    )
                    qn = j % 4
                    if qn:
                        gi.ins.queue = f"qPoolDynamic{qn}"

                # ---- transpose to feature-major: 8 transposes per big tile
                xT = sb.tile([128, batch], f32r, tag="xT", bufs=3, name=f"xT{t}")
                for half in range(2):
                    pT = ps.tile([128, 1024], f32, tag="big", bufs=4,
                                 name=f"pT{t}_{half}")
                    for q in range(8):
                        j = 8 * half + q
                        nc.tensor.transpose(
                            out=pT[:, q * 128:(q + 1) * 128],
                            in_=G[:, j * 128:(j + 1) * 128],
                            identity=ident[:],
                        )
                    nc.scalar.copy(xT[:, half * 1024:(half + 1) * 1024], pT[:])

                rzbuf = sb.tile([128, 2 * batch], f32, tag="rzbuf", bufs=2,
                                name=f"rz{t}")
                narg = sb.tile([128, batch], f32, tag="narg", bufs=2, name=f"na{t}")
                nbuf = sb.tile([128, batch], f32, tag="nbuf", bufs=2, name=f"n{t}")
                dbuf = sb.tile([128, batch], f32, tag="dbuf", bufs=2, name=f"d{t}")
                zd = sb.tile([128, batch], f32, tag="zd", bufs=2, name=f"zd{t}")
                h_new = sb.tile([128, batch], f32r, tag="h", bufs=2, name=f"h{t}")

                rz_tiles = {}
                nxh_tiles = {}
                # ---- gate matmuls, chunk-pair-outer for LDW amortization
                for pair in range(nchunk // 2):
                    cs = (2 * pair, 2 * pair + 1)
                    for c in cs:
                        rz_tiles[c] = ps.tile([128, 1024], f32, tag="big", bufs=4,
                                              name=f"rz{t}_{c}")
                        nxh_tiles[c] = ps.tile([128, 1024], f32, tag="big", bufs=4,
                                               name=f"nxh{t}_{c}")
                    last = t == 0
                    for c in cs:
                        nc.tensor.matmul(out=rz_tiles[c][:, 0:512],
                                         lhsT=wc[:, 0:128],
                                         rhs=xT[:, c * CHUNK:(c + 1) * CHUNK],
                                         start=True, stop=last)
                    for c in cs:
                        nc.tensor.matmul(out=rz_tiles[c][:, 512:1024],
                                         lhsT=wc[:, 128:256],
                                         rhs=xT[:, c * CHUNK:(c + 1) * CHUNK],
                                         start=True, stop=last)
                    for c in cs:
                        nc.tensor.matmul(out=nxh_tiles[c][:, 0:512],
                                         lhsT=wc[:, 256:384],
                                         rhs=xT[:, c * CHUNK:(c + 1) * CHUNK],
                                         start=True, stop=True)
                    if t > 0:
                        for c in cs:
                            nc.tensor.matmul(out=rz_tiles[c][:, 0:512],
                                             lhsT=wh[:, 0:128],
                                             rhs=h_prev[:, c * CHUNK:(c + 1) * CHUNK],
                                             start=False, stop=True)
                        for c in cs:
                            nc.tensor.matmul(out=rz_tiles[c][:, 512:1024],
                                             lhsT=wh[:, 128:256],
                                             rhs=h_prev[:, c * CHUNK:(c + 1) * CHUNK],
                                             start=False, stop=True)
                        for c in cs:
                            nc.tensor.matmul(out=nxh_tiles[c][:, 512:1024],
                                             lhsT=wh[:, 256:384],
                                             rhs=h_prev[:, c * CHUNK:(c + 1) * CHUNK],
                                             start=True, stop=True)

                    # ---- per-chunk gates + h update
                    for c in cs:
                        S = slice(c * CHUNK, (c + 1) * CHUNK)
                        rz_ps = rz_tiles[c]
                        nxh_ps = nxh_tiles[c]
                        if rz_same_bias:
                            out_ap = rzbuf[:].rearrange(
                                "p (g b) -> p g b", g=2)[:, :, S]
                            nc.scalar.activation(out=out_ap, in_=rz_ps[:],
                                                 func=Sig, bias=b_r)
                        else:
                            nc.scalar.activation(out=rzbuf[:, S], in_=rz_ps[:, 0:512],
                                                 func=Sig, bias=b_r)
                            nc.scalar.activation(out=rzbuf[:, batch + c * CHUNK:
                                                           batch + (c + 1) * CHUNK],
                                                 in_=rz_ps[:, 512:1024],
                                                 func=Sig, bias=b_z)
                        r_ap = rzbuf[:, S]
                        z_ap = rzbuf[:, batch + c * CHUNK:batch + (c + 1) * CHUNK]

                        rhn = sb.tile([128, CHUNK], f32, tag="rhn", bufs=2,
                                      name=f"rhn{t}_{c}")
                        if t > 0:
                            nc.vector.scalar_tensor_tensor(
                                out=rhn[:], in0=nxh_ps[:, 512:1024], scalar=b_hn,
                                in1=r_ap, op0=Alu.add, op1=Alu.mult)
                        else:
                            nc.vector.tensor_scalar(
                                out=rhn[:], in0=r_ap, scalar1=b_hn, scalar2=None,
                                op0=Alu.mult)
                        nc.vector.tensor_tensor(out=narg[:, S], in0=rhn[:],
                                                in1=nxh_ps[:, 0:512], op=Alu.add)
                        nc.scalar.activation(out=nbuf[:, S], in_=narg[:, S],
                                             func=Tanh, bias=b_xn)
                        if t > 0:
                            nc.vector.tensor_tensor(out=dbuf[:, S], in0=h_prev[:, S],
                                                    in1=nbuf[:, S], op=Alu.subtract)
                        else:
                            nc.vector.tensor_scalar(out=dbuf[:, S], in0=nbuf[:, S],
                                                    scalar1=-1.0, scalar2=None,
                                                    op0=Alu.mult)
                        nc.vector.tensor_tensor(out=zd[:, S], in0=z_ap,
                                                in1=dbuf[:, S], op=Alu.mult)
                        nc.vector.tensor_tensor(out=h_new[:, S], in0=nbuf[:, S],
                                                in1=zd[:, S], op=Alu.add)

                if debug_taps and t == 0:
                    nc.sync.dma_start(tapG_d[:], G[:])
                    nc.sync.dma_start(tapxT_d[:], xT[:].bitcast(f32))
                    nc.sync.dma_start(tapr_d[:], rzbuf[:, 0:batch])
                    nc.sync.dma_start(tapn_d[:], nbuf[:])
                    nc.sync.dma_start(taph0_d[:], h_new[:].bitcast(f32))
                h_prev = h_new

            nc.sync.dma_start(out_d[:], h_prev[:].bitcast(f32))

    nc.compile()
    return nc


def _get_module(rz_same_bias=True):
    key = ("mod", rz_same_bias)
    if key not in _CACHE:
        _CACHE[key] = _build_module(rz_same_bias=rz_same_bias)
    return _CACHE[key]


def _host_prep(x, walks, W_f, b_f, W_ih, W_hh, b_ih, b_hh):
    """Fold encoder into GRU input projection; pack per-core inputs."""
    x = np.ascontiguousarray(np.asarray(x, dtype=np.float32))
    walks = np.asarray(walks).astype(np.int32)
    W_f = np.asarray(W_f, dtype=np.float32)
    b_f = np.asarray(b_f, dtype=np.float32)
    W_ih = np.asarray(W_ih, dtype=np.float32)
    W_hh = np.asarray(W_hh, dtype=np.float32)
    b_ih = np.asarray(b_ih, dtype=np.float32)
    b_hh = np.asarray(b_hh, dtype=np.float32)

    W_c = (W_ih @ W_f).astype(np.float32)          # [3H, D]
    b_c = (W_ih @ b_f + b_ih).astype(np.float32)   # [3H]

    wct = np.ascontiguousarray(W_c.T)              # [D, 3H] = lhsT layout
    wht = np.ascontiguousarray(W_hh.T)             # [H, 3H]

    # bias columns: b_r, b_z (include recurrent parts), b_hn, b_xn
    bias = np.zeros((128, 4), dtype=np.float32)
    bias[:, 0] = b_c[0:128] + b_hh[0:128]
    bias[:, 1] = b_c[128:256] + b_hh[128:256]
    bias[:, 2] = b_hh[256:384]
    bias[:, 3] = b_c[256:384]

    in_maps = []
    for core in range(NCORES):
        w = walks[core * B:(core + 1) * B]                 # [B, T]
        # idx[p, t*NTILE + j] = walks[j*128 + p, t]
        idx = np.ascontiguousarray(
            w.reshape(NTILE, 128, T).transpose(1, 2, 0).reshape(128, T * NTILE)
        ).astype(np.int32)
        in_maps.append({
            "x": x,
            "idx": idx,
            "wct": wct,
            "wht": wht,
            "bias": bias,
        })
    return in_maps


def kernel(x, walks, W_f, b_f, W_ih, W_hh, b_ih, b_hh):
    from concourse.bass_utils import run_bass_kernel_spmd

    in_maps = _host_prep(x, walks, W_f, b_f, W_ih, W_hh, b_ih, b_hh)
    bias = in_maps[0]["bias"]
    nc = _get_module(rz_same_bias=bool(np.allclose(bias[:, 0], bias[:, 1])))
    res = run_bass_kernel_spmd(nc, in_maps, core_ids=list(range(NCORES)))
    out = np.empty((B_TOTAL, H), dtype=np.float32)
    for core in range(NCORES):
        out[core * B:(core + 1) * B] = res.results[core]["out"].T
    return out


if __name__ == "__main__":
    rng = np.random.default_rng(0)
    ins = {
        "x": rng.standard_normal((N_NODES, D), dtype=np.float32),
        "walks": rng.integers(0, N_NODES, size=(B_TOTAL, T)).astype(np.int32),
        "W_f": rng.standard_normal((H, D), dtype=np.float32) / np.sqrt(D),
        "b_f": np.zeros(H, np.float32),
        "W_ih": rng.standard_normal((3 * H, H), dtype=np.float32) / np.sqrt(H),
        "W_hh": rng.standard_normal((3 * H, H), dtype=np.float32) / np.sqrt(H),
        "b_ih": np.zeros(3 * H, np.float32),
        "b_hh": np.zeros(3 * H, np.float32),
    }
    out = kernel(**ins)
    print(out.shape, out.dtype, np.abs(out).mean())


# revision 13
# speedup vs baseline: 1.2886x; 1.0052x over previous
"""Trainium2 Bass kernel for ExplicitRandomWalkEncoder.

Math (reference):
    x_encoded = x @ W_f.T + b_f                      # [N, H]
    feats     = x_encoded[walks]                     # [B, T, H]
    h_T       = GRU(feats)  (torch gate order r,z,n) # [B, H]

Key algebraic fold: the feature encoder commutes into the GRU input
projection, so the device never materializes x_encoded:
    gx = feats @ W_ih.T + b_ih
       = x[walks] @ (W_ih @ W_f).T + (W_ih @ b_f + b_ih)
The kernel gathers raw x rows (512B each) and applies the combined
input projection W_c = W_ih @ W_f.

Sharding: data-parallel over walks. Each of the 8 cores processes
2048 walks; x and all weights are replicated. No collectives.

Per-core per-step dataflow (hidden-major layout, hidden dim on
partitions, batch on free dim):
  1. indirect DMA gathers 2048 rows of x -> G [128 batch-part, 16*128]
  2. PE transposes each [128,128] walk-tile -> xT [128 feat, 2048]
  3. PE matmuls (float32r): per 512-batch chunk, psum_r/z = W_c_g @ xT
     accumulated with W_hh_g @ h; psum_nx, psum_nh separate
  4. ACT: r = sigmoid(psum_r + b_r), z likewise
  5. DVE: rhn = (psum_nh + b_hn) * r ; narg = rhn + psum_nx
  6. ACT: n = tanh(narg + b_nx)
  7. DVE/Pool: h' = n + z * (h - n)
Output is written hidden-major [128, 2048] per core and transposed
back on the host.
"""

import os

import numpy as np

N_NODES = 200000
D = 128          # input dim
H = 128          # hidden dim
B_TOTAL = 16384  # total walks
T = 20           # walk length
NCORES = 8
B = B_TOTAL // NCORES   # walks per core = 2048
NTILE = B // 128        # 16 walk tiles per step
CHUNK = 512             # batch chunk (one PSUM bank at fp32)
NCHUNK = B // CHUNK     # 4
HALF = B // 2           # elementwise granularity for h-update

_CACHE = {}


def _build_module(n_steps=T, batch=B, debug_taps=False, rz_same_bias=True):
    import concourse.bass as bass
    import concourse.mybir as mybir
    import concourse.tile as tile
    from concourse import bacc
    from concourse.masks import make_identity

    f32 = mybir.dt.float32
    f32r = mybir.dt.float32r
    i32 = mybir.dt.int32
    ntile = batch // 128
    nchunk = batch // CHUNK

    nc = bacc.Bacc(None, target_bir_lowering=False, num_swdge_queues=4)

    x_d = nc.dram_tensor("x", [N_NODES, D], f32, kind="ExternalInput")
    idx_d = nc.dram_tensor("idx", [128, n_steps * ntile], i32, kind="ExternalInput")
    wc_d = nc.dram_tensor("wct", [128, 3 * H], f32r, kind="ExternalInput")
    wh_d = nc.dram_tensor("wht", [128, 3 * H], f32r, kind="ExternalInput")
    b_d = nc.dram_tensor("bias", [128, 4], f32, kind="ExternalInput")
    out_d = nc.dram_tensor("out", [128, batch], f32, kind="ExternalOutput")
    if debug_taps:
        tapG_d = nc.dram_tensor("tapG", [128, batch], f32, kind="ExternalOutput")
        tapxT_d = nc.dram_tensor("tapxT", [128, batch], f32, kind="ExternalOutput")
        tapr_d = nc.dram_tensor("tapr", [128, batch], f32, kind="ExternalOutput")
        tapn_d = nc.dram_tensor("tapn", [128, batch], f32, kind="ExternalOutput")
        taph0_d = nc.dram_tensor("taph0", [128, batch], f32, kind="ExternalOutput")

    Sig = mybir.ActivationFunctionType.Sigmoid
    Tanh = mybir.ActivationFunctionType.Tanh
    Alu = mybir.AluOpType

    with tile.TileContext(nc) as tc:
        with tc.tile_pool(name="cst", bufs=1) as cst, \
             tc.tile_pool(name="sb", bufs=2) as sb, \
             tc.tile_pool(name="ps", bufs=1, space="PSUM") as ps:

            wc = cst.tile([128, 3 * H], f32r, name="wc")
            nc.sync.dma_start(wc[:], wc_d[:])
            wh = cst.tile([128, 3 * H], f32r, name="wh")
            nc.sync.dma_start(wh[:], wh_d[:])
            bias = cst.tile([128, 4], f32, name="biast")
            nc.sync.dma_start(bias[:], b_d[:])
            idx0 = cst.tile([128, n_steps * ntile], i32, name="idxt")
            nc.sync.dma_start(idx0[:], idx_d[:])
            # Pool-engine touch of the index data: forces Q7-visible ordering
            # before any indirect-DMA descriptor generation (stale-read flake
            # seen otherwise on the first gather).
            idx = cst.tile([128, n_steps * ntile], i32, name="idxt2")
            nc.gpsimd.tensor_copy(idx[:], idx0[:])
            ident = cst.tile([128, 128], f32, name="ident")
            make_identity(nc, ident[:])
            scratch = cst.tile([128, 128], f32, name="scratch")
            nc.gpsimd.indirect_dma_start(
                out=scratch[:], out_offset=None, in_=x_d[:],
                in_offset=bass.IndirectOffsetOnAxis(ap=idx[:, 0:1], axis=0))

            b_r = bias[:, 0:1]
            b_z = bias[:, 1:2]
            b_hn = bias[:, 2:3]
            b_xn = bias[:, 3:4]

            h_prev = None
            for t in range(n_steps):
                # ---- gather this step's x rows (one row per partition/instr)
                G = sb.tile([128, batch], f32, tag="G", bufs=5, name=f"G{t}")
                for j in range(ntile):
                    gi = nc.gpsimd.indirect_dma_start(
                        out=G[:, j * 128:(j + 1) * 128],
                        out_offset=None,
                        in_=x_d[:],
                        in_offset=bass.IndirectOffsetOnAxis(
                            ap=idx[:, t * ntile + j:t * ntile + j + 1], axis=0
                        ),
                    )
                    qn = j % 4
                    if qn:
                        gi.ins.queue = f"qPoolDynamic{qn}"

                # ---- transpose to feature-major: 8 transposes per big tile
                xT = sb.tile([128, batch], f32r, tag="xT", bufs=4, name=f"xT{t}")
                for half in range(2):
                    pT = ps.tile([128, 1024], f32, tag="big", bufs=4,
                                 name=f"pT{t}_{half}")
                    for q in range(8):
                        j = 8 * half + q
                        nc.tensor.transpose(
                            out=pT[:, q * 128:(q + 1) * 128],
                            in_=G[:, j * 128:(j + 1) * 128],
                            identity=ident[:],
                        )
                    nc.scalar.copy(xT[:, half * 1024:(half + 1) * 1024], pT[:])

                rzbuf = sb.tile([128, 2 * batch], f32, tag="rzbuf", bufs=2,
                                name=f"rz{t}")
                narg = sb.tile([128, batch], f32, tag="narg", bufs=2, name=f"na{t}")
                nbuf = sb.tile([128, batch], f32, tag="nbuf", bufs=2, name=f"n{t}")
                dbuf = sb.tile([128, batch], f32, tag="dbuf", bufs=2, name=f"d{t}")
                zd = sb.tile([128, batch], f32, tag="zd", bufs=2, name=f"zd{t}")
                h_new = sb.tile([128, batch], f32r, tag="h", bufs=2, name=f"h{t}")

                rz_tiles = {}
                nxh_tiles = {}
                # ---- gate matmuls, chunk-pair-outer for LDW amortization
                for pair in range(nchunk // 2):
                    cs = (2 * pair, 2 * pair + 1)
                    for c in cs:
                        rz_tiles[c] = ps.tile([128, 1024], f32, tag="big", bufs=4,
                                              name=f"rz{t}_{c}")
                        nxh_tiles[c] = ps.tile([128, 1024], f32, tag="big", bufs=4,
                                               name=f"nxh{t}_{c}")
                    last = t == 0
                    for c in cs:
                        nc.tensor.matmul(out=rz_tiles[c][:, 0:512],
                                         lhsT=wc[:, 0:128],
                                         rhs=xT[:, c * CHUNK:(c + 1) * CHUNK],
                                         start=True, stop=last)
                    for c in cs:
                        nc.tensor.matmul(out=rz_tiles[c][:, 512:1024],
                                         lhsT=wc[:, 128:256],
                                         rhs=xT[:, c * CHUNK:(c + 1) * CHUNK],
                                         start=True, stop=last)
                    for c in cs:
                        nc.tensor.matmul(out=nxh_tiles[c][:, 0:512],
                                         lhsT=wc[:, 256:384],
                                         rhs=xT[:, c * CHUNK:(c + 1) * CHUNK],
                                         start=True, stop=True)
                    if t > 0:
                        for c in cs:
                            nc.tensor.matmul(out=rz_tiles[c][:, 0:512],
                                             lhsT=wh[:, 0:128],
                                             rhs=h_prev[:, c * CHUNK:(c + 1) * CHUNK],
                                             start=False, stop=True)
                        for c in cs:
                            nc.tensor.matmul(out=rz_tiles[c][:, 512:1024],
                                             lhsT=wh[:, 128:256],
                                             rhs=h_prev[:, c * CHUNK:(c + 1) * CHUNK],
                                             start=False, stop=True)
                        for c in cs:
                            nc.tensor.matmul(out=nxh_tiles[c][:, 512:1024],
                                             lhsT=wh[:, 256:384],
                                             rhs=h_prev[:, c * CHUNK:(c + 1) * CHUNK],
                                             start=True, stop=True)

                    # ---- per-chunk gates + h update
                    for c in cs:
                        S = slice(c * CHUNK, (c + 1) * CHUNK)
                        rz_ps = rz_tiles[c]
                        nxh_ps = nxh_tiles[c]
                        if rz_same_bias:
                            out_ap = rzbuf[:].rearrange(
                                "p (g b) -> p g b", g=2)[:, :, S]
                            nc.scalar.activation(out=out_ap, in_=rz_ps[:],
                                                 func=Sig, bias=b_r)
                        else:
                            nc.scalar.activation(out=rzbuf[:, S], in_=rz_ps[:, 0:512],
                                                 func=Sig, bias=b_r)
                            nc.scalar.activation(out=rzbuf[:, batch + c * CHUNK:
                                                           batch + (c + 1) * CHUNK],
                                                 in_=rz_ps[:, 512:1024],
                                                 func=Sig, bias=b_z)
                        r_ap = rzbuf[:, S]
                        z_ap = rzbuf[:, batch + c * CHUNK:batch + (c + 1) * CHUNK]

                        rhn = sb.tile([128, CHUNK], f32, tag="rhn", bufs=2,
                                      name=f"rhn{t}_{c}")
                        if t > 0:
                            nc.vector.scalar_tensor_tensor(
                                out=rhn[:], in0=nxh_ps[:, 512:1024], scalar=b_hn,
                                in1=r_ap, op0=Alu.add, op1=Alu.mult)
                        else:
                            nc.vector.tensor_scalar(
                                out=rhn[:], in0=r_ap, scalar1=b_hn, scalar2=None,
                                op0=Alu.mult)
                        nc.vector.tensor_tensor(out=narg[:, S], in0=rhn[:],
                                                in1=nxh_ps[:, 0:512], op=Alu.add)
                        nc.scalar.activation(out=nbuf[:, S], in_=narg[:, S],
                                             func=Tanh, bias=b_xn)
                        if t > 0:
                            nc.vector.tensor_tensor(out=dbuf[:, S], in0=h_prev[:, S],
                                                    in1=nbuf[:, S], op=Alu.subtract)
                        else:
                            nc.vector.tensor_scalar(out=dbuf[:, S], in0=nbuf[:, S],
                                                    scalar1=-1.0, scalar2=None,
                                                    op0=Alu.mult)
                        nc.vector.tensor_tensor(out=zd[:, S], in0=z_ap,
                                                in1=dbuf[:, S], op=Alu.mult)
                        nc.vector.tensor_tensor(out=h_new[:, S], in0=nbuf[:, S],
                                                in1=zd[:, S], op=Alu.add)

                if debug_taps and t == 0:
                    nc.sync.dma_start(tapG_d[:], G[:])
                    nc.sync.dma_start(tapxT_d[:], xT[:].bitcast(f32))
                    nc.sync.dma_start(tapr_d[:], rzbuf[:, 0:batch])
                    nc.sync.dma_start(tapn_d[:], nbuf[:])
                    nc.sync.dma_start(taph0_d[:], h_new[:].bitcast(f32))
                h_prev = h_new

            nc.sync.dma_start(out_d[:], h_prev[:].bitcast(f32))

    nc.compile()
    return nc


def _get_module(rz_same_bias=True):
    key = ("mod", rz_same_bias)
    if key not in _CACHE:
        _CACHE[key] = _build_module(rz_same_bias=rz_same_bias)
    return _CACHE[key]


def _host_prep(x, walks, W_f, b_f, W_ih, W_hh, b_ih, b_hh):
    """Fold encoder into GRU input projection; pack per-core inputs."""
    x = np.ascontiguousarray(np.asarray(x, dtype=np.float32))
    walks = np.asarray(walks).astype(np.int32)
    W_f = np.asarray(W_f, dtype=np.float32)
    b_f = np.asarray(b_f, dtype=np.float32)
    W_ih = np.asarray(W_ih, dtype=np.float32)
    W_hh = np.asarray(W_hh, dtype=np.float32)
    b_ih = np.asarray(b_ih, dtype=np.float32)
    b_hh = np.asarray(b_hh, dtype=np.float32)

    W_c = (W_ih @ W_f).astype(np.float32)          # [3H, D]
    b_c = (W_ih @ b_f + b_ih).astype(np.float32)   # [3H]

    wct = np.ascontiguousarray(W_c.T)              # [D, 3H] = lhsT layout
    wht = np.ascontiguousarray(W_hh.T)             # [H, 3H]

    # bias columns: b_r, b_z (include recurrent parts), b_hn, b_xn
    bias = np.zeros((128, 4), dtype=np.float32)
    bias[:, 0] = b_c[0:128] + b_hh[0:128]
    bias[:, 1] = b_c[128:256] + b_hh[128:256]
    bias[:, 2] = b_hh[256:384]
    bias[:, 3] = b_c[256:384]

    in_maps = []
    for core in range(NCORES):
        w = walks[core * B:(core + 1) * B]                 # [B, T]
        # idx[p, t*NTILE + j] = walks[j*128 + p, t]
        idx = np.ascontiguousarray(
            w.reshape(NTILE, 128, T).transpose(1, 2, 0).reshape(128, T * NTILE)
        ).astype(np.int32)
        in_maps.append({
            "x": x,
            "idx": idx,
            "wct": wct,
            "wht": wht,
            "bias": bias,
        })
    return in_maps


def kernel(x, walks, W_f, b_f, W_ih, W_hh, b_ih, b_hh):
    from concourse.bass_utils import run_bass_kernel_spmd

    in_maps = _host_prep(x, walks, W_f, b_f, W_ih, W_hh, b_ih, b_hh)
    bias = in_maps[0]["bias"]
    nc = _get_module(rz_same_bias=bool(np.allclose(bias[:, 0], bias[:, 1])))
    res = run_bass_kernel_spmd(nc, in_maps, core_ids=list(range(NCORES)))
    out = np.empty((B_TOTAL, H), dtype=np.float32)
    for core in range(NCORES):
        out[core * B:(core + 1) * B] = res.results[core]["out"].T
    return out


if __name__ == "__main__":
    rng = np.random.default_rng(0)
    ins = {
        "x": rng.standard_normal((N_NODES, D), dtype=np.float32),
        "walks": rng.integers(0, N_NODES, size=(B_TOTAL, T)).astype(np.int32),
        "W_f": rng.standard_normal((H, D), dtype=np.float32) / np.sqrt(D),
        "b_f": np.zeros(H, np.float32),
        "W_ih": rng.standard_normal((3 * H, H), dtype=np.float32) / np.sqrt(H),
        "W_hh": rng.standard_normal((3 * H, H), dtype=np.float32) / np.sqrt(H),
        "b_ih": np.zeros(3 * H, np.float32),
        "b_hh": np.zeros(3 * H, np.float32),
    }
    out = kernel(**ins)
    print(out.shape, out.dtype, np.abs(out).mean())
